# revision 4
# baseline (speedup 1.0000x reference)
"""Trainium2 Bass kernel for IrregularDirectionalGradientConv.

Math (per batch element b, channel c, with k = 31, P = 15, L = 961):
    out[c, i, j] = (1/L) * (T^T X_c T)[i, j] - x_pad[c, ci+i, cj+j]
where X_c is the 31x31 image, T[a, b] = 1 iff |a - b| <= 15 (banded ones,
symmetric), and (ci, cj) = divmod(center_idx, 31).  For center_idx = 480
(ci = cj = 15) the center patch is exactly X_c.

Mapping to the PE array: pack 4 channels per 124-partition tile
(partition = 31*c' + h), 8 column-tiles of 31 (free = 31*t + w), channel
c = 4*t + c'.  BD = block_diag(T, T, T, T) [124, 124].  With X as the
*stationary* matmul operand both times the result stays in natural layout
(no transposes):
    U_t = X_t.T @ BD           [(t, w), (c', hout)]    (contract h)
    O_t = (U_t/L).T @ BD       [(c', hout), (t, wout)] (contract w)
    O_t += (-I).T @ Xc_t       (center-patch subtract, PSUM accumulate)
then O is copied PSUM->SBUF (DVE/ACT halves) and DMA'd out.

Each instruction carries at most ONE sem wait (hardware limit): the input
is a single host-packed DMA [x | (xc) | BD | -I], the subtract happens on
the PE, and the two output halves ride the two HWDGE rings independently.
8 batch elements -> 8 NeuronCores, pure data parallel.
"""

import numpy as np

B, C, H, W = 8, 32, 31, 31
KS = 31
P = KS // 2  # 15
L = H * W  # 961

_CACHE = {}


def _consts():
    i = np.arange(KS)
    t = (np.abs(i[:, None] - i[None, :]) <= P).astype(np.float32)
    bd = np.zeros((124, 124), dtype=np.float32)
    for c in range(4):
        bd[31 * c:31 * (c + 1), 31 * c:31 * (c + 1)] = t
    negi = -np.eye(124, dtype=np.float32)
    return np.concatenate([bd, negi], axis=1)  # [124, 248]


def _to_chip(xb):
    """[32, 31, 31] -> [124, 248]: partition 31*c'+h, free 31*t+w, c=4t+c'."""
    return np.ascontiguousarray(
        xb.reshape(8, 4, 31, 31).transpose(1, 2, 0, 3).reshape(124, 248)
    )


def _from_chip(yb):
    """Inverse of _to_chip."""
    return yb.reshape(4, 31, 8, 31).transpose(2, 0, 1, 3).reshape(32, 31, 31)


def _build(has_xc):
    """has_xc: True when the center patch differs from x (center != (15,15))
    and is passed as an extra [124, 248] block in the input."""
    from concourse import bacc, bass, mybir, tile

    f32 = mybir.dt.float32
    nin = 744 if has_xc else 496  # [x | (xc) | bd | negI]
    bd_off = 496 if has_xc else 248
    ni_off = bd_off + 124
    xc_off = 248 if has_xc else 0

    nc = bacc.Bacc(None, target_bir_lowering=False)
    x_d = nc.dram_tensor("xin", [124, nin], f32, kind="ExternalInput")
    y1_d = nc.dram_tensor("y1", [124, 124], f32, kind="ExternalOutput")
    y2_d = nc.dram_tensor("y2", [124, 124], f32, kind="ExternalOutput")

    with tile.TileContext(nc) as tc:
        with (
            tc.tile_pool(name="sb", bufs=1) as sb,
            tc.tile_pool(name="ps", bufs=1, space=bass.MemorySpace.PSUM) as ps,
        ):
            xin = sb.tile([124, nin], f32)
            u1s = sb.tile([124, 124], f32)
            u2s = sb.tile([124, 124], f32)
            res1 = sb.tile([124, 124], f32)
            res2 = sb.tile([124, 124], f32)
            u1 = ps.tile([124, 124], f32)
            u2 = ps.tile([124, 124], f32)
            o1 = ps.tile([124, 124], f32)
            o2 = ps.tile([124, 124], f32)

            nc.sync.dma_start(xin[:], x_d[:])
            bd = xin[:, bd_off:bd_off + 124]
            negi = xin[:, ni_off:ni_off + 124]
            xc1 = xin[:, xc_off:xc_off + 124]
            xc2 = xin[:, xc_off + 124:xc_off + 248]

            inv_l = 1.0 / float(L)
            nc.tensor.matmul(u1[:], xin[:, 0:124], bd, start=True, stop=True)
            nc.tensor.matmul(u2[:], xin[:, 124:248], bd, start=True, stop=True)
            nc.vector.tensor_scalar_mul(u1s[:], u1[:], inv_l)
            nc.scalar.mul(u2s[:], u2[:], inv_l)
            nc.tensor.matmul(o1[:], u1s[:], bd, start=True, stop=False)
            nc.tensor.matmul(o1[:], negi, xc1, start=False, stop=True)
            nc.tensor.matmul(o2[:], u2s[:], bd, start=True, stop=False)
            nc.tensor.matmul(o2[:], negi, xc2, start=False, stop=True)
            nc.vector.tensor_copy(res1[:], o1[:])
            nc.scalar.copy(res2[:], o2[:])
            nc.sync.dma_start(y1_d[:], res1[:])
            nc.scalar.dma_start(y2_d[:], res2[:])

    if not nc.is_finalized():
        nc.finalize()
    return nc


def _get_nc(has_xc):
    if has_xc not in _CACHE:
        _CACHE[has_xc] = _build(has_xc)
    return _CACHE[has_xc]


def _center_patch(xb, ci, cj):
    """[32, 31, 31] -> center patch x_pad[:, ci:ci+31, cj:cj+31]."""
    xp = np.pad(xb, ((0, 0), (P, P), (P, P)))
    return xp[:, ci:ci + KS, cj:cj + KS]


def _run(x, center_idx, trace=False, **kw):
    from concourse.bass_utils import run_bass_kernel_spmd

    ci, cj = divmod(int(center_idx), W)
    has_xc = (ci, cj) != (P, P)
    nc = _get_nc(has_xc)
    x = np.asarray(x, dtype=np.float32)
    assert x.shape == (B, C, H, W)
    consts = _consts()
    in_maps = []
    for b in range(B):
        blocks = [_to_chip(x[b])]
        if has_xc:
            blocks.append(_to_chip(_center_patch(x[b], ci, cj)))
        blocks.append(consts)
        in_maps.append({"xin": np.concatenate(blocks, axis=1)})
    r = run_bass_kernel_spmd(nc, in_maps, list(range(B)), trace=trace, **kw)
    y = np.stack(
        [
            _from_chip(
                np.concatenate([r.results[b]["y1"], r.results[b]["y2"]], axis=1)
            )
            for b in range(B)
        ],
        axis=0,
    )
    return y, r


def kernel(x, center_idx):
    y, _ = _run(x, center_idx, trace=False)
    return y


# revision 5
# speedup vs baseline: 1.0929x; 1.0929x over previous
"""Trainium2 Bass kernel for IrregularDirectionalGradientConv.

Math (per batch element b, channel c, with k = 31, P = 15, L = 961):
    out[c, i, j] = (1/L) * (T^T X_c T)[i, j] - x_pad[c, ci+i, cj+j]
where X_c is the 31x31 image, T[a, b] = 1 iff |a - b| <= 15 (banded ones,
symmetric), and (ci, cj) = divmod(center_idx, 31).

Mapping to the PE array: pack 4 channels per 124-partition tile
(partition = 31*c' + h), 8 column-tiles of 31 (free = 31*t + w), channel
c = 4*t + c'.  BD = block_diag(T, T, T, T) [124, 124].  With X as the
*stationary* matmul operand both times the result stays in natural layout
(no transposes):
    U_t = X_t.T @ BD           [(t, w), (c', hout)]    (contract h)
    O_t = (U_t/L).T @ BD       [(c', hout), (t, wout)] (contract w)
    res = O - Xcenter          (DVE tensor_sub, fp32)

The window-sum matmuls run in bf16 (T exact in bf16, fp32 PSUM accumulate;
only X quantization enters and it is attenuated by the 1/961 mean, ~3e-5
rel) — fp32 matmuls would lower to two quarter-rate passes.  The center
patch is subtracted in full fp32 on the DVE.  The center patch is computed
host-side (np.pad slice), so one program serves every center_idx.

Input rides both HWDGE rings: bf16 [x | BD] on SP (needed first by the
PE), fp32 center patch on ACT.  Every instruction carries at most ONE sem
wait (hardware limit) — a 1-element DVE warm-touch of the fp32 block makes
the DVE observe that DMA early so the final sub needs only the PE wait.
8 batch elements -> 8 NeuronCores, pure data parallel.
"""

import numpy as np

B, C, H, W = 8, 32, 31, 31
KS = 31
P = KS // 2  # 15
L = H * W  # 961

_CACHE = {}


def _bd_const():
    i = np.arange(KS)
    t = (np.abs(i[:, None] - i[None, :]) <= P).astype(np.float32)
    bd = np.zeros((124, 124), dtype=np.float32)
    for c in range(4):
        bd[31 * c:31 * (c + 1), 31 * c:31 * (c + 1)] = t
    return bd


def _to_chip(xb):
    """[32, 31, 31] -> [124, 248]: partition 31*c'+h, free 31*t+w, c=4t+c'."""
    return np.ascontiguousarray(
        xb.reshape(8, 4, 31, 31).transpose(1, 2, 0, 3).reshape(124, 248)
    )


def _from_chip(yb):
    """Inverse of _to_chip."""
    return yb.reshape(4, 31, 8, 31).transpose(2, 0, 1, 3).reshape(32, 31, 31)


def _build():
    from concourse import bacc, bass, mybir, tile

    f32 = mybir.dt.float32
    bf16 = mybir.dt.bfloat16

    nc = bacc.Bacc(None, target_bir_lowering=False)
    xb_d = nc.dram_tensor("xb", [124, 372], bf16, kind="ExternalInput")
    xf_d = nc.dram_tensor("xf", [124, 248], f32, kind="ExternalInput")
    y1_d = nc.dram_tensor("y1", [124, 124], f32, kind="ExternalOutput")
    y2_d = nc.dram_tensor("y2", [124, 124], f32, kind="ExternalOutput")

    with tile.TileContext(nc) as tc:
        with (
            tc.tile_pool(name="sb", bufs=1) as sb,
            tc.tile_pool(name="ps", bufs=1, space=bass.MemorySpace.PSUM) as ps,
        ):
            xbs = sb.tile([124, 372], bf16)
            xfs = sb.tile([124, 248], f32)
            u1s = sb.tile([124, 124], bf16)
            u2s = sb.tile([124, 124], bf16)
            res = sb.tile([124, 248], f32)
            u1 = ps.tile([124, 124], f32)
            u2 = ps.tile([124, 124], f32)
            o = ps.tile([124, 248], f32)

            nc.sync.dma_start(xbs[:], xb_d[:])  # SP ring: bf16 x + BD
            nc.scalar.dma_start(xfs[:], xf_d[:])  # ACT ring: fp32 center patch
            bdb = xbs[:, 248:372]

            inv_l = 1.0 / float(L)
            nc.tensor.matmul(u1[:], xbs[:, 0:124], bdb, start=True, stop=True)
            nc.tensor.matmul(u2[:], xbs[:, 124:248], bdb, start=True, stop=True)
            # Warm-touch: DVE observes the ACT-ring DMA sem here; the WAW dep
            # on u1s orders it before the ts-mul, so the final sub carries
            # only the PE wait.
            nc.vector.tensor_copy(u1s[0:1, 0:1], xfs[0:1, 0:1])
            nc.vector.tensor_scalar_mul(u1s[:], u1[:], inv_l)
            nc.scalar.mul(u2s[:], u2[:], inv_l)
            nc.tensor.matmul(o[:, 0:124], u1s[:], bdb, start=True, stop=True)
            nc.tensor.matmul(o[:, 124:248], u2s[:], bdb, start=True, stop=True)
            nc.vector.tensor_sub(res[:], o[:], xfs[:])
            nc.sync.dma_start(y1_d[:], res[:, 0:124])
            nc.scalar.dma_start(y2_d[:], res[:, 124:248])

    if not nc.is_finalized():
        nc.finalize()
    return nc


def _get_nc():
    if "nc" not in _CACHE:
        _CACHE["nc"] = _build()
    return _CACHE["nc"]


def _center_patch(xb, ci, cj):
    """[32, 31, 31] -> center patch x_pad[:, ci:ci+31, cj:cj+31]."""
    xp = np.pad(xb, ((0, 0), (P, P), (P, P)))
    return xp[:, ci:ci + KS, cj:cj + KS]


def _run(x, center_idx, trace=False, **kw):
    import ml_dtypes
    from concourse.bass_utils import run_bass_kernel_spmd

    ci, cj = divmod(int(center_idx), W)
    nc = _get_nc()
    x = np.asarray(x, dtype=np.float32)
    assert x.shape == (B, C, H, W)
    bd = _bd_const()
    center_is_x = (ci, cj) == (P, P)
    in_maps = []
    for b in range(B):
        xch = _to_chip(x[b])
        xb16 = np.concatenate([xch, bd], axis=1).astype(ml_dtypes.bfloat16)
        xc = xch if center_is_x else _to_chip(_center_patch(x[b], ci, cj))
        in_maps.append({"xb": xb16, "xf": xc})
    r = run_bass_kernel_spmd(nc, in_maps, list(range(B)), trace=trace, **kw)
    y = np.stack(
        [
            _from_chip(
                np.concatenate([r.results[b]["y1"], r.results[b]["y2"]], axis=1)
            )
            for b in range(B)
        ],
        axis=0,
    )
    return y, r


def kernel(x, center_idx):
    y, _ = _run(x, center_idx, trace=False)
    return y


# revision 6
# speedup vs baseline: 1.1719x; 1.0723x over previous
"""Trainium2 Bass kernel for IrregularDirectionalGradientConv.

Math (per batch element b, channel c, with k = 31, P = 15, L = 961):
    out[c, i, j] = (1/L) * (T^T X_c T)[i, j] - x_pad[c, ci+i, cj+j]
where X_c is the 31x31 image, T[a, b] = 1 iff |a - b| <= 15 (banded ones,
symmetric), and (ci, cj) = divmod(center_idx, 31).

Mapping to the PE array: pack 4 channels per 124-partition tile
(partition = 31*c' + h), 8 column-tiles of 31 (free = 31*t + w), channel
c = 4*t + c'.  BD = block_diag(T, T, T, T) [124, 124].  With X as the
*stationary* matmul operand both times the result stays in natural layout
(no transposes):
    U_t = X_t.T @ BD           [(t, w), (c', hout)]    (contract h)
    O_t = (U_t/L).T @ BD       [(c', hout), (t, wout)] (contract w)
    res = O - Xcenter          (DVE tensor_sub, fp32)

The window-sum matmuls run in bf16 (T exact in bf16, fp32 PSUM accumulate;
only X quantization enters and it is attenuated by the 1/961 mean, ~3e-5
rel) — fp32 matmuls would lower to two quarter-rate passes.  The center
patch is subtracted in full fp32 on the DVE.  The center patch is computed
host-side (np.pad slice), so one program serves every center_idx.

Input rides both HWDGE rings: bf16 [x | BD] on SP (needed first by the
PE), fp32 center patch on ACT.  Every instruction carries at most ONE sem
wait (hardware limit) — a 1-element DVE warm-touch of the fp32 block makes
the DVE observe that DMA early so the final sub needs only the PE wait.
8 batch elements -> 8 NeuronCores, pure data parallel.
"""

import numpy as np

B, C, H, W = 8, 32, 31, 31
KS = 31
P = KS // 2  # 15
L = H * W  # 961

_CACHE = {}


def _bd_const():
    i = np.arange(KS)
    t = (np.abs(i[:, None] - i[None, :]) <= P).astype(np.float32)
    bd = np.zeros((124, 124), dtype=np.float32)
    for c in range(4):
        bd[31 * c:31 * (c + 1), 31 * c:31 * (c + 1)] = t
    return bd


def _to_chip(xb):
    """[32, 31, 31] -> [124, 248]: partition 31*c'+h, free 31*t+w, c=4t+c'."""
    return np.ascontiguousarray(
        xb.reshape(8, 4, 31, 31).transpose(1, 2, 0, 3).reshape(124, 248)
    )


def _from_chip(yb):
    """Inverse of _to_chip."""
    return yb.reshape(4, 31, 8, 31).transpose(2, 0, 1, 3).reshape(32, 31, 31)


def _build():
    from concourse import bacc, mybir

    f32 = mybir.dt.float32
    bf16 = mybir.dt.bfloat16

    nc = bacc.Bacc(None, target_bir_lowering=False)
    xb_d = nc.dram_tensor("xb", [124, 372], bf16, kind="ExternalInput")
    xf_d = nc.dram_tensor("xf", [124, 248], f32, kind="ExternalInput")
    y1_d = nc.dram_tensor("y1", [124, 124], f32, kind="ExternalOutput")
    y2_d = nc.dram_tensor("y2", [124, 124], f32, kind="ExternalOutput")

    inv_l = 1.0 / float(L)
    with (
        nc.sbuf_tensor([124, 372], bf16) as xbs,
        nc.sbuf_tensor([124, 248], f32) as xfs,
        nc.sbuf_tensor([124, 124], bf16) as u1s,
        nc.sbuf_tensor([124, 124], bf16) as u2s,
        nc.sbuf_tensor([124, 248], f32) as res,
        nc.psum_tensor([124, 124], f32) as u1,
        nc.psum_tensor([124, 124], f32) as u2,
        nc.psum_tensor([124, 124], f32) as o1,
        nc.psum_tensor([124, 124], f32) as o2,
        nc.semaphore("dma_b") as dma_b,
        nc.semaphore("dma_f") as dma_f,
        nc.semaphore("pe_sem") as pe_sem,
        nc.semaphore("dve_sem") as dve_sem,
        nc.semaphore("act_sem") as act_sem,
        nc.semaphore("out1") as out1,
        nc.semaphore("out2") as out2,
        nc.Block() as block,
    ):
        bdb = xbs[:, 248:372]

        @block.sync
        def _(sync):
            # bf16 [x | BD] — needed first (PE stage A)
            sync.dma_start(out=xbs[:], in_=xb_d[:]).then_inc(dma_b, 16)
            sync.wait_ge(dve_sem, 2)  # sub1 done
            sync.dma_start(out=y1_d[:], in_=res[:, 0:124]).then_inc(out1, 16)
            sync.wait_ge(out1, 16)

        @block.scalar
        def _(scalar):
            # fp32 center patch — needed by DVE sub (~2us later)
            scalar.dma_start(out=xfs[:], in_=xf_d[:]).then_inc(dma_f, 16)
            scalar.wait_ge(pe_sem, 2)
            nc.scalar.mul(u2s[:], u2[:], inv_l).then_inc(act_sem, 1)
            scalar.wait_ge(dve_sem, 3)  # sub2 done
            scalar.dma_start(out=y2_d[:], in_=res[:, 124:248]).then_inc(out2, 16)
            scalar.wait_ge(out2, 16)

        @block.tensor
        def _(tensor):
            tensor.wait_ge(dma_b, 16)
            nc.tensor.matmul(
                u1[:], xbs[:, 0:124], bdb, start=True, stop=True
            ).then_inc(pe_sem, 1)
            nc.tensor.matmul(
                u2[:], xbs[:, 124:248], bdb, start=True, stop=True
            ).then_inc(pe_sem, 1)
            tensor.wait_ge(dve_sem, 1)  # u1s ready
            nc.tensor.matmul(
                o1[:], u1s[:], bdb, start=True, stop=True
            ).then_inc(pe_sem, 1)
            tensor.wait_ge(act_sem, 1)  # u2s ready
            nc.tensor.matmul(
                o2[:], u2s[:], bdb, start=True, stop=True
            ).then_inc(pe_sem, 1)

        @block.vector
        def _(vector):
            vector.wait_ge(pe_sem, 1)
            nc.vector.tensor_scalar_mul(u1s[:], u1[:], inv_l).then_inc(dve_sem, 1)
            vector.wait_ge(pe_sem, 3)  # o1 written
            vector.wait_ge(dma_f, 16)  # xfs loaded
            nc.vector.tensor_sub(
                res[:, 0:124], o1[:], xfs[:, 0:124]
            ).then_inc(dve_sem, 1)
            vector.wait_ge(pe_sem, 4)  # o2 written
            nc.vector.tensor_sub(
                res[:, 124:248], o2[:], xfs[:, 124:248]
            ).then_inc(dve_sem, 1)

    if not nc.is_finalized():
        nc.finalize()
    return nc


def _get_nc():
    if "nc" not in _CACHE:
        _CACHE["nc"] = _build()
    return _CACHE["nc"]


def _center_patch(xb, ci, cj):
    """[32, 31, 31] -> center patch x_pad[:, ci:ci+31, cj:cj+31]."""
    xp = np.pad(xb, ((0, 0), (P, P), (P, P)))
    return xp[:, ci:ci + KS, cj:cj + KS]


def _run(x, center_idx, trace=False, **kw):
    import ml_dtypes
    from concourse.bass_utils import run_bass_kernel_spmd

    ci, cj = divmod(int(center_idx), W)
    nc = _get_nc()
    x = np.asarray(x, dtype=np.float32)
    assert x.shape == (B, C, H, W)
    bd = _bd_const()
    center_is_x = (ci, cj) == (P, P)
    in_maps = []
    for b in range(B):
        xch = _to_chip(x[b])
        xb16 = np.concatenate([xch, bd], axis=1).astype(ml_dtypes.bfloat16)
        xc = xch if center_is_x else _to_chip(_center_patch(x[b], ci, cj))
        in_maps.append({"xb": xb16, "xf": xc})
    r = run_bass_kernel_spmd(nc, in_maps, list(range(B)), trace=trace, **kw)
    y = np.stack(
        [
            _from_chip(
                np.concatenate([r.results[b]["y1"], r.results[b]["y2"]], axis=1)
            )
            for b in range(B)
        ],
        axis=0,
    )
    return y, r


def kernel(x, center_idx):
    y, _ = _run(x, center_idx, trace=False)
    return y


# revision 7
# speedup vs baseline: 1.3225x; 1.1286x over previous
"""Trainium2 Bass kernel for IrregularDirectionalGradientConv.

Math (per batch element b, channel c, with k = 31, P = 15, L = 961):
    out[c, i, j] = (1/L) * (T^T X_c T)[i, j] - x_pad[c, ci+i, cj+j]
where X_c is the 31x31 image, T[a, b] = 1 iff |a - b| <= 15 (banded ones,
symmetric), and (ci, cj) = divmod(center_idx, 31).

Mapping to the PE array: pack 4 channels per 124-partition tile
(partition = 31*c' + h), 8 column-tiles of 31 (free = 31*t + w), channel
c = 4*t + c'.  BD = block_diag(T, T, T, T) [124, 124].  With X as the
*stationary* matmul operand both times the result stays in natural layout
(no transposes):
    U_t = X_t.T @ BD           [(t, w), (c', hout)]    (contract h)
    O_t = (U_t/L).T @ BD       [(c', hout), (t, wout)] (contract w)
    res = O - Xcenter          (DVE tensor_sub, fp32)

The window-sum matmuls run in bf16 (T exact in bf16, fp32 PSUM accumulate;
only X quantization enters and it is attenuated by the 1/961 mean, ~3e-5
rel) — fp32 matmuls would lower to two quarter-rate passes.  The center
patch is subtracted in full fp32 on the DVE.  The center patch is computed
host-side (np.pad slice), so one program serves every center_idx.

Input rides both HWDGE rings: bf16 [x | BD] on SP (needed first by the
PE), fp32 center patch on ACT.  Every instruction carries at most ONE sem
wait (hardware limit) — a 1-element DVE warm-touch of the fp32 block makes
the DVE observe that DMA early so the final sub needs only the PE wait.
8 batch elements -> 8 NeuronCores, pure data parallel.
"""

import numpy as np

B, C, H, W = 8, 32, 31, 31
KS = 31
P = KS // 2  # 15
L = H * W  # 961

_CACHE = {}


def _bd_const():
    i = np.arange(KS)
    t = (np.abs(i[:, None] - i[None, :]) <= P).astype(np.float32)
    bd = np.zeros((124, 124), dtype=np.float32)
    for c in range(4):
        bd[31 * c:31 * (c + 1), 31 * c:31 * (c + 1)] = t
    return bd


def _to_chip(xb):
    """[32, 31, 31] -> [124, 248]: partition 31*c'+h, free 31*t+w, c=4t+c'."""
    return np.ascontiguousarray(
        xb.reshape(8, 4, 31, 31).transpose(1, 2, 0, 3).reshape(124, 248)
    )


def _from_chip(yb):
    """Inverse of _to_chip."""
    return yb.reshape(4, 31, 8, 31).transpose(2, 0, 1, 3).reshape(32, 31, 31)


def _build():
    from concourse import bacc, mybir

    f32 = mybir.dt.float32
    bf16 = mybir.dt.bfloat16

    nc = bacc.Bacc(None, target_bir_lowering=False)
    xb_d = nc.dram_tensor("xb", [124, 372], bf16, kind="ExternalInput")
    xf_d = nc.dram_tensor("xf", [124, 248], f32, kind="ExternalInput")
    y1_d = nc.dram_tensor("y1", [124, 124], f32, kind="ExternalOutput")
    y2_d = nc.dram_tensor("y2", [124, 124], f32, kind="ExternalOutput")

    inv_l = 1.0 / float(L)
    with (
        nc.sbuf_tensor([124, 372], bf16) as xbs,
        nc.sbuf_tensor([124, 248], f32) as xfs,
        nc.sbuf_tensor([124, 124], bf16) as u1s,
        nc.sbuf_tensor([124, 124], bf16) as u2s,
        nc.sbuf_tensor([124, 248], f32) as res,
        nc.psum_tensor([124, 124], f32) as u1,
        nc.psum_tensor([124, 124], f32) as u2,
        nc.psum_tensor([124, 124], f32) as o1,
        nc.psum_tensor([124, 124], f32) as o2,
        nc.semaphore("dma_b") as dma_b,
        nc.semaphore("dma_f") as dma_f,
        nc.semaphore("pe_sem") as pe_sem,
        nc.semaphore("dve_sem") as dve_sem,
        nc.semaphore("act_sem") as act_sem,
        nc.semaphore("out1") as out1,
        nc.semaphore("out2") as out2,
        nc.Block() as block,
    ):
        bdb = xbs[:, 248:372]

        @block.sync
        def _(sync):
            # bf16 [x | BD] first (PE stage A gates on it), then the fp32
            # center patch — serialized so the critical load gets the full
            # SDMA bandwidth; xf is only needed by the DVE subs ~1.5us later.
            sync.dma_start(out=xbs[:], in_=xb_d[:]).then_inc(dma_b, 16)
            sync.dma_start(out=xfs[:], in_=xf_d[:]).then_inc(dma_f, 16)
            sync.wait_ge(dve_sem, 2)  # sub1 done
            sync.dma_start(out=y1_d[:], in_=res[:, 0:124]).then_inc(out1, 16)

        @block.scalar
        def _(scalar):
            scalar.wait_ge(pe_sem, 2)
            nc.scalar.mul(u2s[:], u2[:], inv_l).then_inc(act_sem, 1)
            scalar.wait_ge(dve_sem, 3)  # sub2 done
            scalar.dma_start(out=y2_d[:], in_=res[:, 124:248]).then_inc(out2, 16)

        @block.tensor
        def _(tensor):
            tensor.wait_ge(dma_b, 16)
            nc.tensor.matmul(
                u1[:], xbs[:, 0:124], bdb, start=True, stop=True
            ).then_inc(pe_sem, 1)
            nc.tensor.matmul(
                u2[:], xbs[:, 124:248], bdb, start=True, stop=True
            ).then_inc(pe_sem, 1)
            tensor.wait_ge(dve_sem, 1)  # u1s ready
            nc.tensor.matmul(
                o1[:], u1s[:], bdb, start=True, stop=True
            ).then_inc(pe_sem, 1)
            tensor.wait_ge(act_sem, 1)  # u2s ready
            nc.tensor.matmul(
                o2[:], u2s[:], bdb, start=True, stop=True
            ).then_inc(pe_sem, 1)

        @block.vector
        def _(vector):
            vector.wait_ge(pe_sem, 1)
            nc.vector.tensor_scalar_mul(u1s[:], u1[:], inv_l).then_inc(dve_sem, 1)
            vector.wait_ge(pe_sem, 3)  # o1 written
            vector.wait_ge(dma_f, 16)  # xfs loaded
            nc.vector.tensor_sub(
                res[:, 0:124], o1[:], xfs[:, 0:124]
            ).then_inc(dve_sem, 1)
            vector.wait_ge(pe_sem, 4)  # o2 written
            nc.vector.tensor_sub(
                res[:, 124:248], o2[:], xfs[:, 124:248]
            ).then_inc(dve_sem, 1)

    if not nc.is_finalized():
        nc.finalize()
    return nc


def _get_nc():
    if "nc" not in _CACHE:
        _CACHE["nc"] = _build()
    return _CACHE["nc"]


def _center_patch(xb, ci, cj):
    """[32, 31, 31] -> center patch x_pad[:, ci:ci+31, cj:cj+31]."""
    xp = np.pad(xb, ((0, 0), (P, P), (P, P)))
    return xp[:, ci:ci + KS, cj:cj + KS]


def _run(x, center_idx, trace=False, **kw):
    import ml_dtypes
    from concourse.bass_utils import run_bass_kernel_spmd

    ci, cj = divmod(int(center_idx), W)
    nc = _get_nc()
    x = np.asarray(x, dtype=np.float32)
    assert x.shape == (B, C, H, W)
    bd = _bd_const()
    center_is_x = (ci, cj) == (P, P)
    in_maps = []
    for b in range(B):
        xch = _to_chip(x[b])
        xb16 = np.concatenate([xch, bd], axis=1).astype(ml_dtypes.bfloat16)
        xc = xch if center_is_x else _to_chip(_center_patch(x[b], ci, cj))
        in_maps.append({"xb": xb16, "xf": xc})
    r = run_bass_kernel_spmd(nc, in_maps, list(range(B)), trace=trace, **kw)
    y = np.stack(
        [
            _from_chip(
                np.concatenate([r.results[b]["y1"], r.results[b]["y2"]], axis=1)
            )
            for b in range(B)
        ],
        axis=0,
    )
    return y, r


def kernel(x, center_idx):
    y, _ = _run(x, center_idx, trace=False)
    return y


# revision 8
# speedup vs baseline: 1.3488x; 1.0199x over previous
"""Trainium2 Bass kernel for IrregularDirectionalGradientConv.

Math (per batch element b, channel c, with k = 31, P = 15, L = 961):
    out[c, i, j] = (1/L) * (T^T X_c T)[i, j] - x_pad[c, ci+i, cj+j]
where X_c is the 31x31 image, T[a, b] = 1 iff |a - b| <= 15 (banded ones,
symmetric), and (ci, cj) = divmod(center_idx, 31).

Mapping to the PE array: pack 4 channels per 124-partition tile
(partition = 31*c' + h), 8 column-tiles of 31 (free = 31*t + w), channel
c = 4*t + c'.  BD = block_diag(T, T, T, T) [124, 124].  With X as the
*stationary* matmul operand both times the result stays in natural layout
(no transposes):
    U_t = X_t.T @ BD           [(t, w), (c', hout)]    (contract h)
    O_t = (U_t/L).T @ BD       [(c', hout), (t, wout)] (contract w)
    res = O - Xcenter          (DVE tensor_sub, fp32)

The window-sum matmuls run in bf16 (T exact in bf16, fp32 PSUM accumulate;
only X quantization enters and it is attenuated by the 1/961 mean, ~3e-5
rel) — fp32 matmuls would lower to two quarter-rate passes.  The center
patch is subtracted in full fp32 on the DVE.  The center patch is computed
host-side (np.pad slice), so one program serves every center_idx.

Input rides both HWDGE rings: bf16 [x | BD] on SP (needed first by the
PE), fp32 center patch on ACT.  Every instruction carries at most ONE sem
wait (hardware limit) — a 1-element DVE warm-touch of the fp32 block makes
the DVE observe that DMA early so the final sub needs only the PE wait.
8 batch elements -> 8 NeuronCores, pure data parallel.
"""

import numpy as np

B, C, H, W = 8, 32, 31, 31
KS = 31
P = KS // 2  # 15
L = H * W  # 961

_CACHE = {}


def _bd_const():
    i = np.arange(KS)
    t = (np.abs(i[:, None] - i[None, :]) <= P).astype(np.float32)
    bd = np.zeros((124, 124), dtype=np.float32)
    for c in range(4):
        bd[31 * c:31 * (c + 1), 31 * c:31 * (c + 1)] = t
    return bd


def _to_chip(xb):
    """[32, 31, 31] -> [124, 248]: partition 31*c'+h, free 31*t+w, c=4t+c'."""
    return np.ascontiguousarray(
        xb.reshape(8, 4, 31, 31).transpose(1, 2, 0, 3).reshape(124, 248)
    )


def _from_chip(yb):
    """Inverse of _to_chip."""
    return yb.reshape(4, 31, 8, 31).transpose(2, 0, 1, 3).reshape(32, 31, 31)


def _build():
    from concourse import bacc, mybir

    f32 = mybir.dt.float32
    bf16 = mybir.dt.bfloat16

    nc = bacc.Bacc(None, target_bir_lowering=False)
    xb_d = nc.dram_tensor("xb", [124, 372], bf16, kind="ExternalInput")
    xf_d = nc.dram_tensor("xf", [124, 248], f32, kind="ExternalInput")
    y1_d = nc.dram_tensor("y1", [124, 124], f32, kind="ExternalOutput")
    y2_d = nc.dram_tensor("y2", [124, 124], f32, kind="ExternalOutput")

    inv_l = 1.0 / float(L)
    with (
        nc.sbuf_tensor([124, 372], bf16) as xbs,
        nc.sbuf_tensor([124, 248], f32) as xfs,
        nc.sbuf_tensor([124, 124], bf16) as u1s,
        nc.sbuf_tensor([124, 124], bf16) as u2s,
        nc.sbuf_tensor([124, 248], f32) as res,
        nc.psum_tensor([124, 124], f32) as u1,
        nc.psum_tensor([124, 124], f32) as u2,
        nc.psum_tensor([124, 124], f32) as o1,
        nc.psum_tensor([124, 124], f32) as o2,
        nc.semaphore("dma_b") as dma_b,
        nc.semaphore("dma_f") as dma_f,
        nc.semaphore("pe_sem") as pe_sem,
        nc.semaphore("dve_sem") as dve_sem,
        nc.semaphore("act_sem") as act_sem,
        nc.semaphore("out1") as out1,
        nc.semaphore("out2") as out2,
        nc.Block() as block,
    ):
        bdb = xbs[:, 248:372]

        @block.sync
        def _(sync):
            # bf16 [x | BD] first (PE stage A gates on it), then the fp32
            # center patch — serialized so the critical load gets the full
            # SDMA bandwidth; xf is only needed by the DVE subs ~1.5us later.
            sync.dma_start(out=xbs[:], in_=xb_d[:]).then_inc(dma_b, 16)
            sync.dma_start(out=xfs[:], in_=xf_d[:]).then_inc(dma_f, 16)
            sync.wait_ge(dve_sem, 2)  # sub1 done
            sync.dma_start(out=y1_d[:], in_=res[:, 0:124]).then_inc(out1, 16)
            sync.wait_ge(dve_sem, 3)  # sub2 done
            sync.dma_start(out=y2_d[:], in_=res[:, 124:248]).then_inc(out2, 16)

        @block.scalar
        def _(scalar):
            scalar.wait_ge(pe_sem, 2)
            nc.scalar.mul(u2s[:], u2[:], inv_l).then_inc(act_sem, 1)

        @block.tensor
        def _(tensor):
            tensor.wait_ge(dma_b, 16)
            nc.tensor.matmul(
                u1[:], xbs[:, 0:124], bdb, start=True, stop=True
            ).then_inc(pe_sem, 1)
            nc.tensor.matmul(
                u2[:], xbs[:, 124:248], bdb, start=True, stop=True
            ).then_inc(pe_sem, 1)
            tensor.wait_ge(dve_sem, 1)  # u1s ready
            nc.tensor.matmul(
                o1[:], u1s[:], bdb, start=True, stop=True
            ).then_inc(pe_sem, 1)
            tensor.wait_ge(act_sem, 1)  # u2s ready
            nc.tensor.matmul(
                o2[:], u2s[:], bdb, start=True, stop=True
            ).then_inc(pe_sem, 1)

        @block.vector
        def _(vector):
            vector.wait_ge(pe_sem, 1)
            nc.vector.tensor_scalar_mul(u1s[:], u1[:], inv_l).then_inc(dve_sem, 1)
            vector.wait_ge(pe_sem, 3)  # o1 written
            vector.wait_ge(dma_f, 16)  # xfs loaded
            nc.vector.tensor_sub(
                res[:, 0:124], o1[:], xfs[:, 0:124]
            ).then_inc(dve_sem, 1)
            vector.wait_ge(pe_sem, 4)  # o2 written
            nc.vector.tensor_sub(
                res[:, 124:248], o2[:], xfs[:, 124:248]
            ).then_inc(dve_sem, 1)

    if not nc.is_finalized():
        nc.finalize()
    return nc


def _get_nc():
    if "nc" not in _CACHE:
        _CACHE["nc"] = _build()
    return _CACHE["nc"]


def _center_patch(xb, ci, cj):
    """[32, 31, 31] -> center patch x_pad[:, ci:ci+31, cj:cj+31]."""
    xp = np.pad(xb, ((0, 0), (P, P), (P, P)))
    return xp[:, ci:ci + KS, cj:cj + KS]


def _run(x, center_idx, trace=False, **kw):
    import ml_dtypes
    from concourse.bass_utils import run_bass_kernel_spmd

    ci, cj = divmod(int(center_idx), W)
    nc = _get_nc()
    x = np.asarray(x, dtype=np.float32)
    assert x.shape == (B, C, H, W)
    bd = _bd_const()
    center_is_x = (ci, cj) == (P, P)
    in_maps = []
    for b in range(B):
        xch = _to_chip(x[b])
        xb16 = np.concatenate([xch, bd], axis=1).astype(ml_dtypes.bfloat16)
        xc = xch if center_is_x else _to_chip(_center_patch(x[b], ci, cj))
        in_maps.append({"xb": xb16, "xf": xc})
    r = run_bass_kernel_spmd(nc, in_maps, list(range(B)), trace=trace, **kw)
    y = np.stack(
        [
            _from_chip(
                np.concatenate([r.results[b]["y1"], r.results[b]["y2"]], axis=1)
            )
            for b in range(B)
        ],
        axis=0,
    )
    return y, r


def kernel(x, center_idx):
    y, _ = _run(x, center_idx, trace=False)
    return y
